# revision 14
# baseline (speedup 1.0000x reference)
"""CLIP-style loss kernel for Trainium2 (8 NeuronCores, SPMD data-parallel).

Problem: two patch-embeddings (stride-4 4x4 conv -> L2 normalize) of
imgs/hha [32,64,128,128], per-sample logits = exp(logit_scale) * a @ h^T
[B,1024,1024], symmetric cross-entropy with diagonal labels, scalar loss.

Sharding: data-parallel over batch, 4 samples per core. Each core reads only
its 4x2 images; produces per-sample partial sums (row-lse, col-lse, diag);
host combines in float64.

Per-core pipeline (v3):
  conv:   fp8(e4m3) images+weights (weights pre-scaled x64 for subnormal
          hygiene; the scale cancels through the normalization), DoubleRow
          matmuls (K=256 per chunk) into 1-bank PSUM tiles on a dedicated
          pool so conv can fill PE gaps while the logits pool cycles.
  norms:  both modalities' norm^2 in T-layout [n-part, chunk] via FD=1
          matmuls (lhsT = sq chunk, bf16 -> FWL), copied to SBUF to free
          the PSUM bank early. rsqrt on DVE via the convert-based quake
          trick + 1 Newton iter (NO ACT Ln/Exp -> the ACT table stays on
          the exp set for the whole kernel body).
  h_hat:  invh [128,8] -> PE transpose -> [8,128] -> selector matmuls
          broadcast (2 halves through the conv pool) -> h_hat = ym_h * bc.
  logits: per chunk k: 2 matmuls -> L [128,1024] (2-bank tiles, bufs=2);
          ONE exp (FD=1024) with the a-side row scale via the ACT
          per-partition scale operand and the row-sum via accum_out.
  colsum: csum += E on DVE (bf16, 2x mode); T-reduce via FD=1 matmuls.
  diag:   t = ya .* h_hat; per-chunk column sums via FD=1 matmuls; dot with
          invT.
  lse:    deferred: row-sums and col-sums for ALL samples staged in SBUF;
          two Ln ops at kernel end with accum_out -> OUT.
  pipeline: 3-stage software pipeline -- conv_stage(b+2) || tail_stage(b+1)
          || consume(b) -- so PE always has conv work while ACT drains exps.
Output per core: [128, 2 + BPC] partial-sum block; host reduces.
"""

import os
import sys
from contextlib import ExitStack

import numpy as np

for _p in ("/opt/trn_rl_repo", "/root/.axon_site/_ro/trn_rl_repo"):
    if os.path.isdir(_p) and _p not in sys.path:
        sys.path.insert(0, _p)

import concourse.bass as bass
import concourse.mybir as mybir
import concourse.tile as tile
from concourse import bacc
from concourse.bass_utils import run_bass_kernel_spmd

F32 = mybir.dt.float32
I32 = mybir.dt.int32
BF16 = mybir.dt.bfloat16
FP8 = mybir.dt.float8e4
AF = mybir.ActivationFunctionType
ALU = mybir.AluOpType
DR = mybir.MatmulPerfMode.DoubleRow

N_CORES = 8
B_FULL = 32
BPC = B_FULL // N_CORES  # samples per core
C, H, W, D, P = 64, 128, 128, 128, 4
NPAT = (H // P) * (W // P)  # 1024 patches
NH = NPAT // 2  # 512 (one patch-half / one PSUM bank)
NOFF = P * P  # 16 kernel offsets
NCHUNK = NPAT // 128  # 8 logit row chunks
NKC2 = (C * NOFF) // 256  # 4 conv contraction chunks (K=256, DoubleRow)

QUAKE_C = 1597463007.0  # 0x5f3759df as a float value
WSCALE = 64.0  # fp8 weight pre-scale (cancels through normalization)


def build_program(ln_s: float) -> bass.Bass:
    nc = bacc.Bacc(None)
    s2inv = float(np.exp(-2.0 * ln_s))  # 1/s^2

    imgs = nc.declare_dram_parameter(
        "imgs", [BPC, 128, NKC2, 2, NPAT], FP8, isOutput=False
    )
    hha = nc.declare_dram_parameter(
        "hha", [BPC, 128, NKC2, 2, NPAT], FP8, isOutput=False
    )
    w1t = nc.declare_dram_parameter("w1t", [128, NKC2, 2, D], FP8, isOutput=False)
    w2t = nc.declare_dram_parameter("w2t", [128, NKC2, 2, D], FP8, isOutput=False)
    b1 = nc.declare_dram_parameter("b1", [D], F32, isOutput=False)
    b2 = nc.declare_dram_parameter("b2", [D], F32, isOutput=False)
    ident_d = nc.declare_dram_parameter("ident", [128, 128], BF16, isOutput=False)
    sel_d = nc.declare_dram_parameter("sel", [8, NCHUNK * 128], BF16, isOutput=False)
    out_d = nc.declare_dram_parameter("out", [128, 2 + BPC], F32, isOutput=True)

    srcs = (imgs, hha)

    with tile.TileContext(nc) as tc, ExitStack() as ctx:
        # SBUF pools
        p_img = ctx.enter_context(tc.tile_pool(name="img", bufs=4))
        p_one = ctx.enter_context(tc.tile_pool(name="singles", bufs=1))
        p_ysb = ctx.enter_context(tc.tile_pool(name="ysb", bufs=6))
        p_sq = ctx.enter_context(tc.tile_pool(name="sq", bufs=4))
        p_hhat = ctx.enter_context(tc.tile_pool(name="hhat", bufs=2))
        p_E = ctx.enter_context(tc.tile_pool(name="E", bufs=4))
        p_cs = ctx.enter_context(tc.tile_pool(name="cs", bufs=2))
        p_sm = ctx.enter_context(tc.tile_pool(name="small", bufs=2))
        p_n2 = ctx.enter_context(tc.tile_pool(name="n2", bufs=3))
        # PSUM pools (8 banks: conv/bc 2x1 + logits 2x2 + T 2x1)
        pp_c = ctx.enter_context(tc.tile_pool(name="ppc", bufs=2, space="PSUM"))
        pp_L = ctx.enter_context(tc.tile_pool(name="ppL", bufs=2, space="PSUM"))
        pp_T = ctx.enter_context(tc.tile_pool(name="ppT", bufs=2, space="PSUM"))

        # constants / weights (loaded once)
        ones_k = p_one.tile([128, 1], BF16)
        nc.vector.memset(ones_k, 1.0)
        ident = p_one.tile([128, 128], BF16)
        nc.sync.dma_start(out=ident, in_=ident_d[:])
        sel = p_one.tile([8, NCHUNK * 128], BF16)
        nc.sync.dma_start(out=sel, in_=sel_d[:])
        wts = []
        biases = []
        for wsrc, bsrc in ((w1t, b1), (w2t, b2)):
            wt = p_one.tile([128, NKC2, 2, D], FP8, tag=f"wt_{wsrc.name}")
            nc.sync.dma_start(out=wt, in_=wsrc[:])
            wts.append(wt)
            bt = p_one.tile([128, 1], F32, tag=f"bias_{bsrc.name}")
            nc.sync.dma_start(out=bt, in_=bsrc[:].rearrange("(d one) -> d one", one=1))
            biases.append(bt)
        OUT = p_one.tile([128, 2 + BPC], F32)
        nc.vector.memset(OUT, 0.0)
        # scm: quake input scale (col 0-7: 1/s^2 for the a-side, 8-15: 1.0)
        scm = p_one.tile([128, 16], F32)
        nc.vector.memset(scm[:, 0:8], s2inv)
        nc.vector.memset(scm[:, 8:16], 1.0)
        # persistent per-sample stages
        invTH = p_one.tile([128, 16 * BPC], F32)  # [invT | invh] per sample
        RS_all = p_one.tile([128, NCHUNK * BPC], F32)  # exp row sums
        csT_all = p_one.tile([128, NCHUNK * BPC], F32)  # exp col sums (T)

        # Warmup matmuls absorb the weight-DMA waits into PE program order.
        wu = pp_T.tile([128, 16], F32, tag="T", name="wu")
        for m in range(2):
            nc.tensor.matmul(
                wu[:, m : m + 1],
                wts[m][:, 0],
                wts[m][:, 0, :, 0:1],
                start=True,
                stop=True,
                perf_mode=DR,
            )

        def conv_work(b, out):
            """Generator: load + conv + bias + sq + T-norm^2 for sample b.
            Yields between quanta so the driver can interleave emission."""
            y_sb = {}
            n2c = pp_T.tile([128, 16], F32, tag="T", name=f"n2c_{b}")
            for m in range(2):
                img = p_img.tile([128, NKC2, 2, NPAT], FP8, tag="img")
                if b == 0:
                    # split so the first conv group starts after ~1/4 of the
                    # transfer (kernel-startup latency)
                    for kc in range(NKC2):
                        nc.sync.dma_start(out=img[:, kc], in_=srcs[m][b][:, kc])
                else:
                    nc.sync.dma_start(out=img, in_=srcs[m][b])

                ym = p_ysb.tile([128, NPAT], BF16, tag="ysb")
                sq = p_sq.tile([128, NPAT], BF16, tag="sq")
                for t in range(2):
                    Y = pp_c.tile([128, NH], F32, tag="c", name=f"cv_{b}_{m}_{t}")
                    for kc in range(NKC2):
                        nc.tensor.matmul(
                            Y,
                            wts[m][:, kc],
                            img[:, kc, :, t * NH : (t + 1) * NH],
                            start=(kc == 0),
                            stop=(kc == NKC2 - 1),
                            perf_mode=DR,
                        )
                    nc.vector.tensor_scalar_add(
                        ym[:, t * NH : (t + 1) * NH], Y, biases[m]
                    )
                    nc.vector.tensor_mul(
                        sq[:, t * NH : (t + 1) * NH],
                        ym[:, t * NH : (t + 1) * NH],
                        ym[:, t * NH : (t + 1) * NH],
                    )
                    yield
                y_sb[m] = ym
                for k0 in range(0, NCHUNK, 4):
                    for k in range(k0, k0 + 4):
                        nc.tensor.matmul(
                            n2c[:, m * 8 + k : m * 8 + k + 1],
                            sq[:, 128 * k : 128 * (k + 1)],
                            ones_k,
                            start=True,
                            stop=True,
                        )
                    yield
            n2s = p_n2.tile([128, 16], F32, tag="n2")
            nc.vector.tensor_copy(n2s, n2c)  # free the PSUM bank early
            out["y"] = y_sb
            out["n2"] = n2s

        def tail_work(b, cs, out):
            """Generator: rsqrt + h_hat for sample b."""
            n2s, y_sb = cs["n2"], cs["y"]
            # quake rsqrt on [128,16] (cols 0-7: a-side with s folded; 8-15: h)
            qx = p_sm.tile([128, 16], F32, tag="qx")
            nc.vector.tensor_mul(qx, n2s, scm)
            qf = p_sm.tile([128, 16], F32, tag="qf")
            nc.vector.tensor_copy(qf, qx[:].bitcast(I32))  # int-value as float
            qi = p_sm.tile([128, 16], I32, tag="qi")
            nc.vector.tensor_scalar(
                qi, qf, -0.5, QUAKE_C, op0=ALU.mult, op1=ALU.add
            )
            yield
            y0 = qi[:].bitcast(F32)
            qt = p_sm.tile([128, 16], F32, tag="qt")
            nc.vector.tensor_mul(qt, y0, y0)
            nc.vector.tensor_mul(qt, qt, qx)
            nc.vector.tensor_scalar(qt, qt, -0.5, 1.5, op0=ALU.mult, op1=ALU.add)
            inv = invTH[:, 16 * b : 16 * (b + 1)]
            nc.vector.tensor_mul(inv, y0, qt)
            yield
            # h_hat: invh -> bf16 -> PE transpose -> selector broadcast -> mul
            ihb = p_sm.tile([128, 8], BF16, tag="ihb")
            nc.vector.tensor_copy(ihb, inv[:, 8:16])
            ih8 = pp_T.tile([8, 128], BF16, tag="T", name=f"ih8_{b}")
            nc.tensor.transpose(ih8, ihb, ident)
            ih8s = p_sm.tile([8, 128], BF16, tag="ih8s")
            nc.vector.tensor_copy(ih8s, ih8)
            yield
            h_hat = p_hhat.tile([128, NPAT], BF16, tag="hhat")
            for t in range(2):
                bc = pp_c.tile([128, NH], F32, tag="c", name=f"bc_{b}_{t}")
                for q in range(4):
                    qq = 4 * t + q
                    nc.tensor.matmul(
                        bc[:, 128 * q : 128 * (q + 1)],
                        sel[:, 128 * qq : 128 * (qq + 1)],
                        ih8s,
                        start=True,
                        stop=True,
                    )
                nc.vector.tensor_mul(
                    h_hat[:, t * NH : (t + 1) * NH],
                    y_sb[1][:, t * NH : (t + 1) * NH],
                    bc,
                )
                yield
            out["y"] = y_sb
            out["h"] = h_hat

        def fin_work(b, st):
            """Generator: T-layout partial sums for sample b (after its
            csum/t_ are complete); runs inside iteration b+1's exp window."""
            csum, t_ = st["cs"], st["t"]
            ct16 = pp_T.tile([128, 16], F32, tag="T", name=f"ct16_{b}")
            for k0 in range(0, NCHUNK, 4):
                for k in range(k0, k0 + 4):
                    nc.tensor.matmul(
                        ct16[:, 8 + k : 9 + k],
                        t_[:, 128 * k : 128 * (k + 1)],
                        ones_k,
                        start=True,
                        stop=True,
                    )
                yield
            for k0 in range(0, NCHUNK, 4):
                for k in range(k0, k0 + 4):
                    nc.tensor.matmul(
                        ct16[:, k : k + 1],
                        csum[:, 128 * k : 128 * (k + 1)],
                        ones_k,
                        start=True,
                        stop=True,
                    )
                yield
            nc.vector.tensor_copy(
                csT_all[:, NCHUNK * b : NCHUNK * (b + 1)], ct16[:, 0:8]
            )
            dg = p_sm.tile([128, NCHUNK], F32, tag="dg")
            nc.vector.tensor_mul(
                dg, ct16[:, 8:16], invTH[:, 16 * b : 16 * b + 8]
            )
            nc.vector.tensor_reduce(
                out=OUT[:, 2 + b : 3 + b],
                in_=dg,
                axis=mybir.AxisListType.X,
                op=ALU.add,
            )

        def drain(gens, n=1):
            """Advance each live generator up to n quanta."""
            for g in list(gens):
                for _ in range(n):
                    try:
                        next(g)
                    except StopIteration:
                        gens.remove(g)
                        break

        def consume(b, st, side):
            """Logits + exp + csum for sample b, interleaving side work.
            The E-sum is split: E0-E3 accumulate on GpSimd (otherwise idle),
            E4-E7 on DVE; one final DVE add merges."""
            ya, h_hat = st["y"][0], st["h"]
            t_ = p_sq.tile([128, NPAT], BF16, tag="sq")
            nc.gpsimd.tensor_mul(t_, ya, h_hat)

            csA = p_cs.tile([128, NPAT], BF16, tag="csA")
            csum = p_cs.tile([128, NPAT], BF16, tag="cs")
            Es = {}
            for k in range(NCHUNK):
                L = pp_L.tile([128, NPAT], F32, tag="L", name=f"L_{b}_{k}")
                for j in range(2):
                    nc.tensor.matmul(
                        L[:, j * NH : (j + 1) * NH],
                        ya[:, 128 * k : 128 * (k + 1)],
                        h_hat[:, j * NH : (j + 1) * NH],
                        start=True,
                        stop=True,
                    )
                E = p_E.tile([128, NPAT], BF16, tag="E", name=f"E_{b}_{k}")
                Es[k] = E
                nc.scalar.activation(
                    out=E,
                    in_=L,
                    func=AF.Exp,
                    scale=invTH[:, 16 * b + k : 16 * b + k + 1],
                    accum_out=RS_all[:, NCHUNK * b + k : NCHUNK * b + k + 1],
                )
                if k == 1:
                    nc.gpsimd.tensor_add(csA, Es[0], Es[1])
                elif k in (2, 3):
                    nc.gpsimd.tensor_add(csA, csA, E)
                elif k == 5:
                    nc.vector.tensor_add(csum, Es[4], Es[5])
                elif k > 5:
                    nc.vector.tensor_add(csum, csum, E)
                drain(side, 1)
            nc.vector.tensor_add(csum, csum, csA)
            return {"cs": csum, "t": t_}

        # 3-stage interleaved software pipeline
        outs = {b: {} for b in range(BPC)}
        tout = {b: {} for b in range(BPC)}
        for _ in conv_work(0, outs[0]):
            pass
        if BPC > 1:
            for _ in conv_work(1, outs[1]):
                pass
        for _ in tail_work(0, outs[0], tout[0]):
            pass
        fin_prev = None
        for b in range(BPC):
            side = []
            if fin_prev is not None:
                side.append(fin_prev)
            if b + 2 < BPC:
                side.append(conv_work(b + 2, outs[b + 2]))
            if b + 1 < BPC:
                side.append(tail_work(b + 1, outs[b + 1], tout[b + 1]))
            st = consume(b, tout[b], side)
            drain(side, 100)  # finish any leftovers
            fin_prev = fin_work(b, st)
        for _ in fin_prev:
            pass

        # deferred lse finalize (the only ACT table switch of the kernel)
        lnR = p_sm.tile([128, NCHUNK * BPC], F32, tag="lnR")
        nc.scalar.activation(
            out=lnR, in_=RS_all, func=AF.Ln, accum_out=OUT[:, 0:1]
        )
        lnC = p_sm.tile([128, NCHUNK * BPC], F32, tag="lnC")
        nc.scalar.activation(
            out=lnC, in_=csT_all, func=AF.Ln, accum_out=OUT[:, 1:2]
        )
        nc.sync.dma_start(out=out_d[:], in_=OUT)

    nc.compile()
    return nc


_PROGRAM_CACHE: dict = {}


def _get_program(ln_s: float) -> bass.Bass:
    key = round(float(ln_s), 9)
    if key not in _PROGRAM_CACHE:
        _PROGRAM_CACHE[key] = build_program(float(ln_s))
    return _PROGRAM_CACHE[key]


def make_in_maps(imgs, hha, w1, b1, w2, b2):
    """Shard full inputs into per-core input maps (host-side, cheap)."""
    import ml_dtypes

    bf16 = ml_dtypes.bfloat16
    fp8 = ml_dtypes.float8_e4m3

    def prep_w(w):
        # [D,C,P,P] -> [(c,di,dj)=1024, D] -> [feat%128, chunk, ko, D] fp8 x64
        wf = np.transpose(np.asarray(w), (1, 2, 3, 0)).reshape(C * NOFF, D)
        wf = np.clip(wf * WSCALE, -240.0, 240.0)
        return np.ascontiguousarray(
            wf.reshape(NKC2, 2, 128, D).transpose(2, 0, 1, 3)
        ).astype(fp8)

    def prep_x(x):
        # stride==kernel -> im2col is a permutation:
        # [B,C,H,W] -> [B, (c,di,dj)=1024, (i,j)=1024] -> [B,128,NKC2,2,NPAT]
        B = x.shape[0]
        xp = np.asarray(x).reshape(B, C, H // P, P, W // P, P)
        xp = xp.transpose(0, 1, 3, 5, 2, 4).reshape(B, C * NOFF, NPAT)
        xp = np.clip(xp, -240.0, 240.0)
        return np.ascontiguousarray(
            xp.reshape(B, NKC2, 2, 128, NPAT).transpose(0, 3, 1, 2, 4)
        ).astype(fp8)

    w1t = prep_w(w1)
    w2t = prep_w(w2)
    imgs = prep_x(imgs)
    hha = prep_x(hha)
    b1 = np.ascontiguousarray(np.asarray(b1) * WSCALE, dtype=np.float32)
    b2 = np.ascontiguousarray(np.asarray(b2) * WSCALE, dtype=np.float32)
    ident = np.eye(128, dtype=bf16)
    sel = np.zeros((8, NCHUNK * 128), dtype=bf16)
    for q in range(NCHUNK):
        sel[q, 128 * q : 128 * (q + 1)] = 1.0
    maps = []
    for i in range(N_CORES):
        maps.append(
            {
                "imgs": np.ascontiguousarray(imgs[i * BPC : (i + 1) * BPC]),
                "hha": np.ascontiguousarray(hha[i * BPC : (i + 1) * BPC]),
                "w1t": w1t,
                "w2t": w2t,
                "b1": b1,
                "b2": b2,
                "ident": ident,
                "sel": sel,
            }
        )
    return maps


def combine_outputs(outs) -> np.float32:
    """Reduce the 8 per-core [128, 2+BPC] partial blocks to the scalar loss."""
    tot = np.float64(0.0)
    for o in outs:
        o = np.asarray(o, dtype=np.float64)
        lse_row = o[:, 0].sum()
        lse_col = o[:, 1].sum()
        diag = o[:, 2 : 2 + BPC].sum()
        tot += 0.5 * (lse_row + lse_col) - diag
    return np.float32(tot / (B_FULL * NPAT))


def run_spmd(imgs, hha, w1, b1, w2, b2, logit_scale, **kwargs):
    """Run on the 8 cores; returns (loss, BassKernelResults)."""
    ln_s = float(np.asarray(logit_scale))
    nc = _get_program(ln_s)
    in_maps = make_in_maps(imgs, hha, w1, b1, w2, b2)
    res = run_bass_kernel_spmd(nc, in_maps, list(range(N_CORES)), **kwargs)
    return combine_outputs([r["out"] for r in res.results]), res


def kernel(imgs, hha, w1, b1, w2, b2, logit_scale):
    loss, _ = run_spmd(imgs, hha, w1, b1, w2, b2, logit_scale)
    return loss


if __name__ == "__main__":
    # smoke test against a tiny numpy reference of the math
    rng = np.random.default_rng(0)
    imgs = rng.standard_normal((B_FULL, C, H, W), dtype=np.float32)
    hha = rng.standard_normal((B_FULL, C, H, W), dtype=np.float32)
    w1 = rng.standard_normal((D, C, P, P), dtype=np.float32) * 0.03
    w2 = rng.standard_normal((D, C, P, P), dtype=np.float32) * 0.03
    b1 = np.zeros(D, np.float32)
    b2 = np.zeros(D, np.float32)
    ls = np.float32(np.log(1.0 / 0.07))
    print(kernel(imgs, hha, w1, b1, w2, b2, ls))


# revision 17
# speedup vs baseline: 1.1073x; 1.1073x over previous
"""CLIP-style loss kernel for Trainium2 (8 NeuronCores, SPMD data-parallel).

Problem: two patch-embeddings (stride-4 4x4 conv -> L2 normalize) of
imgs/hha [32,64,128,128], per-sample logits = exp(logit_scale) * a @ h^T
[B,1024,1024], symmetric cross-entropy with diagonal labels, scalar loss.

Sharding: data-parallel over batch, 4 samples per core. Each core reads only
its 4x2 images; produces per-sample partial sums (row-lse, col-lse, diag);
host combines in float64.

Per-core pipeline (v3):
  conv:   fp8(e4m3) images+weights (weights pre-scaled x64 for subnormal
          hygiene; the scale cancels through the normalization), DoubleRow
          matmuls (K=256 per chunk) into 1-bank PSUM tiles on a dedicated
          pool so conv can fill PE gaps while the logits pool cycles.
  norms:  both modalities' norm^2 in T-layout [n-part, chunk] via FD=1
          matmuls (lhsT = sq chunk, bf16 -> FWL), copied to SBUF to free
          the PSUM bank early. rsqrt on DVE via the convert-based quake
          trick + 1 Newton iter (NO ACT Ln/Exp -> the ACT table stays on
          the exp set for the whole kernel body).
  h_hat:  invh [128,8] -> PE transpose -> [8,128] -> selector matmuls
          broadcast (2 halves through the conv pool) -> h_hat = ym_h * bc.
  logits: per chunk k: 2 matmuls -> L [128,1024] (2-bank tiles, bufs=2);
          ONE exp (FD=1024) with the a-side row scale via the ACT
          per-partition scale operand and the row-sum via accum_out.
  colsum: csum += E on DVE (bf16, 2x mode); T-reduce via FD=1 matmuls.
  diag:   t = ya .* h_hat; per-chunk column sums via FD=1 matmuls; dot with
          invT.
  lse:    deferred: row-sums and col-sums for ALL samples staged in SBUF;
          two Ln ops at kernel end with accum_out -> OUT.
  pipeline: 3-stage software pipeline -- conv_stage(b+2) || tail_stage(b+1)
          || consume(b) -- so PE always has conv work while ACT drains exps.
Output per core: [128, 2 + BPC] partial-sum block; host reduces.
"""

import os
import sys
from contextlib import ExitStack

import numpy as np

for _p in ("/opt/trn_rl_repo", "/root/.axon_site/_ro/trn_rl_repo"):
    if os.path.isdir(_p) and _p not in sys.path:
        sys.path.insert(0, _p)

import concourse.bass as bass
import concourse.mybir as mybir
import concourse.tile as tile
from concourse import bacc
from concourse.bass_utils import run_bass_kernel_spmd

F32 = mybir.dt.float32
I32 = mybir.dt.int32
BF16 = mybir.dt.bfloat16
FP8 = mybir.dt.float8e4
AF = mybir.ActivationFunctionType
ALU = mybir.AluOpType
DR = mybir.MatmulPerfMode.DoubleRow

N_CORES = 8
B_FULL = 32
BPC = B_FULL // N_CORES  # samples per core
C, H, W, D, P = 64, 128, 128, 128, 4
NPAT = (H // P) * (W // P)  # 1024 patches
NH = NPAT // 2  # 512 (one patch-half / one PSUM bank)
NOFF = P * P  # 16 kernel offsets
NCHUNK = NPAT // 128  # 8 logit row chunks
NKC2 = (C * NOFF) // 256  # 4 conv contraction chunks (K=256, DoubleRow)

QUAKE_C = 1597463007.0  # 0x5f3759df as a float value
WSCALE = 64.0  # fp8 weight pre-scale (cancels through normalization)


def build_program(ln_s: float) -> bass.Bass:
    nc = bacc.Bacc(None)
    s2inv = float(np.exp(-2.0 * ln_s))  # 1/s^2

    imgs = nc.declare_dram_parameter(
        "imgs", [BPC, 128, NKC2, 2, NPAT], FP8, isOutput=False
    )
    hha = nc.declare_dram_parameter(
        "hha", [BPC, 128, NKC2, 2, NPAT], FP8, isOutput=False
    )
    w1t = nc.declare_dram_parameter("w1t", [128, NKC2, 2, D], FP8, isOutput=False)
    w2t = nc.declare_dram_parameter("w2t", [128, NKC2, 2, D], FP8, isOutput=False)
    b1 = nc.declare_dram_parameter("b1", [D], F32, isOutput=False)
    b2 = nc.declare_dram_parameter("b2", [D], F32, isOutput=False)
    ident_d = nc.declare_dram_parameter("ident", [128, 128], BF16, isOutput=False)
    sel_d = nc.declare_dram_parameter("sel", [8, NCHUNK * 128], BF16, isOutput=False)
    out_d = nc.declare_dram_parameter("out", [128, 2 + BPC], F32, isOutput=True)

    srcs = (imgs, hha)

    with tile.TileContext(nc) as tc, ExitStack() as ctx:
        # SBUF pools
        p_img = ctx.enter_context(tc.tile_pool(name="img", bufs=4))
        p_one = ctx.enter_context(tc.tile_pool(name="singles", bufs=1))
        p_ysb = ctx.enter_context(tc.tile_pool(name="ysb", bufs=6))
        p_sq = ctx.enter_context(tc.tile_pool(name="sq", bufs=4))
        p_hhat = ctx.enter_context(tc.tile_pool(name="hhat", bufs=2))
        p_E = ctx.enter_context(tc.tile_pool(name="E", bufs=4))
        p_cs = ctx.enter_context(tc.tile_pool(name="cs", bufs=2))
        p_sm = ctx.enter_context(tc.tile_pool(name="small", bufs=2))
        p_n2 = ctx.enter_context(tc.tile_pool(name="n2", bufs=3))
        # PSUM pools (8 banks: conv/bc 2x1 + logits 2x2 + T 2x1)
        pp_c = ctx.enter_context(tc.tile_pool(name="ppc", bufs=2, space="PSUM"))
        pp_L = ctx.enter_context(tc.tile_pool(name="ppL", bufs=2, space="PSUM"))
        pp_T = ctx.enter_context(tc.tile_pool(name="ppT", bufs=2, space="PSUM"))

        # weights first on the sync queue (the first conv waits on them);
        # small constants go via SWDGE (gpsimd) to keep sync free for images
        wts = []
        biases = []
        for wsrc, bsrc in ((w1t, b1), (w2t, b2)):
            wt = p_one.tile([128, NKC2, 2, D], FP8, tag=f"wt_{wsrc.name}")
            nc.sync.dma_start(out=wt, in_=wsrc[:])
            wts.append(wt)
            bt = p_one.tile([128, 1], F32, tag=f"bias_{bsrc.name}")
            nc.gpsimd.dma_start(out=bt, in_=bsrc[:].rearrange("(d one) -> d one", one=1))
            biases.append(bt)
        ones_k = p_one.tile([128, 1], BF16)
        nc.vector.memset(ones_k, 1.0)
        ident = p_one.tile([128, 128], BF16)
        nc.gpsimd.dma_start(out=ident, in_=ident_d[:])
        sel = p_one.tile([8, NCHUNK * 128], BF16)
        nc.gpsimd.dma_start(out=sel, in_=sel_d[:])
        OUT = p_one.tile([128, 2 + BPC], F32)
        nc.vector.memset(OUT, 0.0)
        # scm: quake input scale (col 0-7: 1/s^2 for the a-side, 8-15: 1.0)
        scm = p_one.tile([128, 16], F32)
        nc.vector.memset(scm[:, 0:8], s2inv)
        nc.vector.memset(scm[:, 8:16], 1.0)
        # persistent per-sample stages
        invTH = p_one.tile([128, 16 * BPC], F32)  # [invT | invh] per sample
        # row sums (cols 0..31) and col sums (cols 32..63) for all samples;
        # one deferred Ln+accum covers both (host scales by 0.5)
        RSCS = p_one.tile([128, 2 * NCHUNK * BPC], F32)

        # Warmup matmuls absorb the weight-DMA waits into PE program order.
        wu = pp_T.tile([128, 16], F32, tag="T", name="wu")
        for m in range(2):
            nc.tensor.matmul(
                wu[:, m : m + 1],
                wts[m][:, 0],
                wts[m][:, 0, :, 0:1],
                start=True,
                stop=True,
                perf_mode=DR,
            )

        def conv_work(b, out):
            """Generator: load + conv + bias + sq + T-norm^2 for sample b.
            Yields between quanta so the driver can interleave emission."""
            y_sb = {}
            n2c = pp_T.tile([128, 16], F32, tag="T", name=f"n2c_{b}")
            for m in range(2):
                img = p_img.tile([128, NKC2, 2, NPAT], FP8, tag="img")
                if b == 0:
                    # split so the first conv group starts after ~1/4 of the
                    # transfer (kernel-startup latency)
                    for kc in range(NKC2):
                        nc.sync.dma_start(out=img[:, kc], in_=srcs[m][b][:, kc])
                else:
                    nc.sync.dma_start(out=img, in_=srcs[m][b])

                ym = p_ysb.tile([128, NPAT], BF16, tag="ysb")
                sq = p_sq.tile([128, NPAT], BF16, tag="sq")
                for t in range(2):
                    Y = pp_c.tile([128, NH], F32, tag="c", name=f"cv_{b}_{m}_{t}")
                    for kc in range(NKC2):
                        nc.tensor.matmul(
                            Y,
                            wts[m][:, kc],
                            img[:, kc, :, t * NH : (t + 1) * NH],
                            start=(kc == 0),
                            stop=(kc == NKC2 - 1),
                            perf_mode=DR,
                        )
                        if kc == 1:
                            yield
                    nc.vector.tensor_scalar_add(
                        ym[:, t * NH : (t + 1) * NH], Y, biases[m]
                    )
                    nc.vector.tensor_mul(
                        sq[:, t * NH : (t + 1) * NH],
                        ym[:, t * NH : (t + 1) * NH],
                        ym[:, t * NH : (t + 1) * NH],
                    )
                    yield
                y_sb[m] = ym
                for k0 in range(0, NCHUNK, 4):
                    for k in range(k0, k0 + 4):
                        nc.tensor.matmul(
                            n2c[:, m * 8 + k : m * 8 + k + 1],
                            sq[:, 128 * k : 128 * (k + 1)],
                            ones_k,
                            start=True,
                            stop=True,
                        )
                    yield
            n2s = p_n2.tile([128, 16], F32, tag="n2")
            nc.vector.tensor_copy(n2s, n2c)  # free the PSUM bank early
            out["y"] = y_sb
            out["n2"] = n2s

        def tail_work(b, cs, out):
            """Generator: rsqrt + h_hat for sample b."""
            n2s, y_sb = cs["n2"], cs["y"]
            # quake rsqrt on [128,16] (cols 0-7: a-side with s folded; 8-15: h)
            qx = p_sm.tile([128, 16], F32, tag="qx")
            nc.vector.tensor_mul(qx, n2s, scm)
            qf = p_sm.tile([128, 16], F32, tag="qf")
            nc.vector.tensor_copy(qf, qx[:].bitcast(I32))  # int-value as float
            qi = p_sm.tile([128, 16], I32, tag="qi")
            nc.vector.tensor_scalar(
                qi, qf, -0.5, QUAKE_C, op0=ALU.mult, op1=ALU.add
            )
            yield
            y0 = qi[:].bitcast(F32)
            qt = p_sm.tile([128, 16], F32, tag="qt")
            nc.vector.tensor_mul(qt, y0, y0)
            nc.vector.tensor_mul(qt, qt, qx)
            nc.vector.tensor_scalar(qt, qt, -0.5, 1.5, op0=ALU.mult, op1=ALU.add)
            inv = invTH[:, 16 * b : 16 * (b + 1)]
            nc.vector.tensor_mul(inv, y0, qt)
            yield
            # h_hat: invh -> bf16 -> PE transpose -> selector broadcast -> mul
            ihb = p_sm.tile([128, 8], BF16, tag="ihb")
            nc.vector.tensor_copy(ihb, inv[:, 8:16])
            ih8 = pp_T.tile([8, 128], BF16, tag="T", name=f"ih8_{b}")
            nc.tensor.transpose(ih8, ihb, ident)
            ih8s = p_sm.tile([8, 128], BF16, tag="ih8s")
            nc.vector.tensor_copy(ih8s, ih8)
            yield
            h_hat = p_hhat.tile([128, NPAT], BF16, tag="hhat")
            for t in range(2):
                bc = pp_c.tile([128, NH], F32, tag="c", name=f"bc_{b}_{t}")
                for q in range(4):
                    qq = 4 * t + q
                    nc.tensor.matmul(
                        bc[:, 128 * q : 128 * (q + 1)],
                        sel[:, 128 * qq : 128 * (qq + 1)],
                        ih8s,
                        start=True,
                        stop=True,
                    )
                nc.vector.tensor_mul(
                    h_hat[:, t * NH : (t + 1) * NH],
                    y_sb[1][:, t * NH : (t + 1) * NH],
                    bc,
                )
                yield
            out["y"] = y_sb
            out["h"] = h_hat

        def fin_work(b, st):
            """Generator: T-layout partial sums for sample b (after its
            csum/t_ are complete); runs inside iteration b+1's exp window."""
            csum, t_ = st["cs"], st["t"]
            ct16 = pp_T.tile([128, 16], F32, tag="T", name=f"ct16_{b}")
            for k0 in range(0, NCHUNK, 4):
                for k in range(k0, k0 + 4):
                    nc.tensor.matmul(
                        ct16[:, 8 + k : 9 + k],
                        t_[:, 128 * k : 128 * (k + 1)],
                        ones_k,
                        start=True,
                        stop=True,
                    )
                yield
            for k0 in range(0, NCHUNK, 4):
                for k in range(k0, k0 + 4):
                    nc.tensor.matmul(
                        ct16[:, k : k + 1],
                        csum[:, 128 * k : 128 * (k + 1)],
                        ones_k,
                        start=True,
                        stop=True,
                    )
                yield
            base = NCHUNK * (BPC + b)
            nc.vector.tensor_copy(RSCS[:, base : base + NCHUNK], ct16[:, 0:8])
            dg = p_sm.tile([128, NCHUNK], F32, tag="dg")
            nc.vector.tensor_mul(
                dg, ct16[:, 8:16], invTH[:, 16 * b : 16 * b + 8]
            )
            nc.vector.tensor_reduce(
                out=OUT[:, 2 + b : 3 + b],
                in_=dg,
                axis=mybir.AxisListType.X,
                op=ALU.add,
            )

        def drain(gens, n=1):
            """Advance each live generator up to n quanta."""
            for g in list(gens):
                for _ in range(n):
                    try:
                        next(g)
                    except StopIteration:
                        gens.remove(g)
                        break

        def consume(b, st, side):
            """Logits + exp + csum for sample b, interleaving side work.
            The E-sum is split: E0-E3 accumulate on GpSimd (otherwise idle),
            E4-E7 on DVE; one final DVE add merges."""
            ya, h_hat = st["y"][0], st["h"]
            t_ = p_sq.tile([128, NPAT], BF16, tag="sq")
            nc.vector.tensor_mul(t_, ya, h_hat)

            csum = p_cs.tile([128, NPAT], BF16, tag="cs")
            Es = {}
            for k in range(NCHUNK):
                L = pp_L.tile([128, NPAT], F32, tag="L", name=f"L_{b}_{k}")
                for j in range(2):
                    nc.tensor.matmul(
                        L[:, j * NH : (j + 1) * NH],
                        ya[:, 128 * k : 128 * (k + 1)],
                        h_hat[:, j * NH : (j + 1) * NH],
                        start=True,
                        stop=True,
                    )
                E = p_E.tile([128, NPAT], BF16, tag="E", name=f"E_{b}_{k}")
                Es[k] = E
                nc.scalar.activation(
                    out=E,
                    in_=L,
                    func=AF.Exp,
                    scale=invTH[:, 16 * b + k : 16 * b + k + 1],
                    accum_out=RSCS[:, NCHUNK * b + k : NCHUNK * b + k + 1],
                )
                if k == 1:
                    nc.vector.tensor_add(csum, Es[0], Es[1])
                elif k > 1:
                    nc.vector.tensor_add(csum, csum, E)
                drain(side, 1)
            return {"cs": csum, "t": t_}

        # 3-stage interleaved software pipeline
        outs = {b: {} for b in range(BPC)}
        tout = {b: {} for b in range(BPC)}
        for _ in conv_work(0, outs[0]):
            pass
        if BPC > 1:
            for _ in conv_work(1, outs[1]):
                pass
        for _ in tail_work(0, outs[0], tout[0]):
            pass
        fin_prev = None
        for b in range(BPC):
            side = []
            if fin_prev is not None:
                side.append(fin_prev)
            if b + 2 < BPC:
                side.append(conv_work(b + 2, outs[b + 2]))
            if b + 1 < BPC:
                side.append(tail_work(b + 1, outs[b + 1], tout[b + 1]))
            st = consume(b, tout[b], side)
            drain(side, 100)  # finish any leftovers
            fin_prev = fin_work(b, st)
        for _ in fin_prev:
            pass

        # deferred lse finalize (the only ACT table switch of the kernel)
        lnRC = p_sm.tile([128, 2 * NCHUNK * BPC], F32, tag="lnRC")
        nc.scalar.activation(
            out=lnRC, in_=RSCS, func=AF.Ln, accum_out=OUT[:, 0:1]
        )
        nc.sync.dma_start(out=out_d[:], in_=OUT)

    nc.compile()
    return nc


_PROGRAM_CACHE: dict = {}


def _get_program(ln_s: float) -> bass.Bass:
    key = round(float(ln_s), 9)
    if key not in _PROGRAM_CACHE:
        _PROGRAM_CACHE[key] = build_program(float(ln_s))
    return _PROGRAM_CACHE[key]


def make_in_maps(imgs, hha, w1, b1, w2, b2):
    """Shard full inputs into per-core input maps (host-side, cheap)."""
    import ml_dtypes

    bf16 = ml_dtypes.bfloat16
    fp8 = ml_dtypes.float8_e4m3

    def prep_w(w):
        # [D,C,P,P] -> [(c,di,dj)=1024, D] -> [feat%128, chunk, ko, D] fp8 x64
        wf = np.transpose(np.asarray(w), (1, 2, 3, 0)).reshape(C * NOFF, D)
        wf = np.clip(wf * WSCALE, -240.0, 240.0)
        return np.ascontiguousarray(
            wf.reshape(NKC2, 2, 128, D).transpose(2, 0, 1, 3)
        ).astype(fp8)

    def prep_x(x):
        # stride==kernel -> im2col is a permutation:
        # [B,C,H,W] -> [B, (c,di,dj)=1024, (i,j)=1024] -> [B,128,NKC2,2,NPAT]
        B = x.shape[0]
        xp = np.asarray(x).reshape(B, C, H // P, P, W // P, P)
        xp = xp.transpose(0, 1, 3, 5, 2, 4).reshape(B, C * NOFF, NPAT)
        xp = np.clip(xp, -240.0, 240.0)
        return np.ascontiguousarray(
            xp.reshape(B, NKC2, 2, 128, NPAT).transpose(0, 3, 1, 2, 4)
        ).astype(fp8)

    w1t = prep_w(w1)
    w2t = prep_w(w2)
    imgs = prep_x(imgs)
    hha = prep_x(hha)
    b1 = np.ascontiguousarray(np.asarray(b1) * WSCALE, dtype=np.float32)
    b2 = np.ascontiguousarray(np.asarray(b2) * WSCALE, dtype=np.float32)
    ident = np.eye(128, dtype=bf16)
    sel = np.zeros((8, NCHUNK * 128), dtype=bf16)
    for q in range(NCHUNK):
        sel[q, 128 * q : 128 * (q + 1)] = 1.0
    maps = []
    for i in range(N_CORES):
        maps.append(
            {
                "imgs": np.ascontiguousarray(imgs[i * BPC : (i + 1) * BPC]),
                "hha": np.ascontiguousarray(hha[i * BPC : (i + 1) * BPC]),
                "w1t": w1t,
                "w2t": w2t,
                "b1": b1,
                "b2": b2,
                "ident": ident,
                "sel": sel,
            }
        )
    return maps


def combine_outputs(outs) -> np.float32:
    """Reduce the 8 per-core [128, 2+BPC] partial blocks to the scalar loss."""
    tot = np.float64(0.0)
    for o in outs:
        o = np.asarray(o, dtype=np.float64)
        lse_rc = o[:, 0].sum()
        diag = o[:, 2 : 2 + BPC].sum()
        tot += 0.5 * lse_rc - diag
    return np.float32(tot / (B_FULL * NPAT))


def run_spmd(imgs, hha, w1, b1, w2, b2, logit_scale, **kwargs):
    """Run on the 8 cores; returns (loss, BassKernelResults)."""
    ln_s = float(np.asarray(logit_scale))
    nc = _get_program(ln_s)
    in_maps = make_in_maps(imgs, hha, w1, b1, w2, b2)
    res = run_bass_kernel_spmd(nc, in_maps, list(range(N_CORES)), **kwargs)
    return combine_outputs([r["out"] for r in res.results]), res


def kernel(imgs, hha, w1, b1, w2, b2, logit_scale):
    loss, _ = run_spmd(imgs, hha, w1, b1, w2, b2, logit_scale)
    return loss


if __name__ == "__main__":
    # smoke test against a tiny numpy reference of the math
    rng = np.random.default_rng(0)
    imgs = rng.standard_normal((B_FULL, C, H, W), dtype=np.float32)
    hha = rng.standard_normal((B_FULL, C, H, W), dtype=np.float32)
    w1 = rng.standard_normal((D, C, P, P), dtype=np.float32) * 0.03
    w2 = rng.standard_normal((D, C, P, P), dtype=np.float32) * 0.03
    b1 = np.zeros(D, np.float32)
    b2 = np.zeros(D, np.float32)
    ls = np.float32(np.log(1.0 / 0.07))
    print(kernel(imgs, hha, w1, b1, w2, b2, ls))
